# revision 1
# baseline (speedup 1.0000x reference)
"""MoE FFN (grouped sigmoid top-k routing + shared expert) on 8 TRN2 NeuronCores.

Strategy: expert-parallel. Each core gets 2 of 16 routed experts plus 1/8 of
the shared expert (sharded along its hidden dim HS). x is replicated
(host-pre-transposed to [C, S] so every matmul contracts over the SBUF
partition dim). Routing is computed on-device, replicated on every core.
Each core emits a partial output [C, S]; the host sums the 8 partials and
transposes back.

dtypes: router matmuls run in full fp32 (top-k selection is sensitive to
input rounding); FFN matmuls run in fp32r (fp32 rounded to 11 mantissa bits,
full PE rate, ~1e-4 relative error).
"""

import numpy as np

import concourse.bacc as bacc
import concourse.mybir as mybir
from concourse import tile
from concourse.bass_utils import run_bass_kernel_spmd
from concourse.masks import make_identity

F32 = mybir.dt.float32
F32R = mybir.dt.float32r
AF = mybir.ActivationFunctionType
OP = mybir.AluOpType

# problem shapes (hardcoded; kernel.py must be self-contained)
B, T, C, H, HS = 2, 1024, 1024, 256, 2048
E, G, EPG = 16, 4, 4
TOPK = 4
NCORES = 8
S = B * T                  # 2048 tokens
EPC = E // NCORES          # 2 experts per core
HSL = HS // NCORES         # 256 shared-hidden rows per core
KC = C // 128              # 8 contraction chunks
NT = S // 128              # 16 token chunks
NSC = S // 512             # 4 moving (token) chunks of 512
NHC = H // 128             # 2 h chunks (same for HSL)
NCC = C // 128             # 8 output-row chunks


def _round_f32r(x: np.ndarray) -> np.ndarray:
    """Round fp32 to fp32r (RNE to 11 mantissa bits) — matches TRN2 PE."""
    u = np.ascontiguousarray(x, dtype=np.float32).view(np.uint32)
    u = u + 0x7FF + ((u >> 12) & 1)
    u = u & np.uint32(0xFFFFF000)
    return u.view(np.float32)


def build():
    nc = bacc.Bacc(
        "TRN2",
        target_bir_lowering=False,
        debug=False,
        enable_asserts=True,
        num_devices=NCORES,
    )
    # ---- DRAM I/O (per core) ----
    x_d = nc.declare_dram_parameter("xT", [C, S], F32, isOutput=False)
    rw_d = nc.declare_dram_parameter("rw", [128, 128], F32, isOutput=False)
    bias_d = nc.declare_dram_parameter("bias", [1, E], F32, isOutput=False)
    esel_d = nc.declare_dram_parameter("esel", [E, EPC * 128], F32R,
                                       isOutput=False)
    gw_d = nc.declare_dram_parameter("gw", [EPC, C, H], F32R, isOutput=False)
    uw_d = nc.declare_dram_parameter("uw", [EPC, C, H], F32R, isOutput=False)
    dw_d = nc.declare_dram_parameter("dw", [EPC, H, C], F32R, isOutput=False)
    sgw_d = nc.declare_dram_parameter("sgw", [C, HSL], F32R, isOutput=False)
    suw_d = nc.declare_dram_parameter("suw", [C, HSL], F32R, isOutput=False)
    sdw_d = nc.declare_dram_parameter("sdw", [HSL, C], F32R, isOutput=False)
    out_d = nc.declare_dram_parameter("out", [C, S], F32, isOutput=True)

    with tile.TileContext(nc) as tc:
        _emit(nc, tc, x_d, rw_d, bias_d, esel_d, gw_d, uw_d, dw_d,
              sgw_d, suw_d, sdw_d, out_d)
    nc.finalize()
    return nc


def _emit(nc, tc, x_d, rw_d, bias_d, esel_d, gw_d, uw_d, dw_d,
          sgw_d, suw_d, sdw_d, out_d):
    consts = tc.alloc_tile_pool(name="consts", bufs=1)
    ident = consts.tile([128, 128], F32)
    make_identity(nc, ident[:])
    rw = consts.tile([128, 128], F32)
    nc.sync.dma_start(rw[:], rw_d[:])
    bias_sb = consts.tile([1, E], F32)
    nc.sync.dma_start(bias_sb[:], bias_d[:])
    esel = consts.tile([E, EPC * 128], F32R)
    nc.sync.dma_start(esel[:], esel_d[:])
    # down-proj weights, resident (all 3 sources needed together in the
    # down pass): wd[src][p, hc*1024 + c] = dw[src][hc*128+p, c]
    wd = [consts.tile([128, NHC * C], F32R, tag=f"wd{i}", name=f"wd{i}")
          for i in range(3)]
    comb = consts.tile([128, NT * E], F32)       # combine weights [s, (t e)]

    # hw tiles [128, S] fp32r: (src, hc) -> silu(g)*u (* combine weight)
    hw_pool = tc.alloc_tile_pool(name="hw", bufs=1)
    hw = [[hw_pool.tile([128, S], F32R, tag=f"hw{src}{hc}",
                        name=f"hw{src}{hc}")
           for hc in range(NHC)] for src in range(3)]

    # x_r: fp32r copy of x, resident for all FFN matmuls
    xr_pool = tc.alloc_tile_pool(name="xr", bufs=1)
    x_r = xr_pool.tile([128, KC * S], F32R)

    # gate/up weight pool (opened early so expert 0's weights stream in
    # behind the first x chunk, during the router phase)
    wp = tc.alloc_tile_pool(name="wp", bufs=2)
    w_tiles = {}

    def load_w(src):
        # one [128, KC*128] tile per (proj, hc): finer slot rotation lets the
        # next source's first-half weights stream while the current source is
        # still computing its second half
        tiles = {}
        for proj, wsrc in (("g", gw_d[src] if src < 2 else sgw_d),
                           ("u", uw_d[src] if src < 2 else suw_d)):
            for hc in range(NHC):
                wt = wp.tile([128, KC * 128], F32R, tag=f"{proj}{hc}",
                             name=f"w{proj}{src}{hc}")
                nc.sync.dma_start(
                    wt.rearrange("p (k h) -> p k h", k=KC),
                    wsrc.rearrange("(k p) h -> p k h", p=128)[
                        :, :, hc * 128:(hc + 1) * 128])
                tiles[(proj, hc)] = wt
        w_tiles[src] = tiles

    # ---------------- router + routing (scoped pools) ----------------
    with (
        tc.tile_pool(name="rt", bufs=1) as rt,
        tc.tile_pool(name="xs", bufs=2) as xs,
        tc.tile_pool(name="psl", bufs=NSC, space="PSUM") as psl,
        tc.tile_pool(name="pst", bufs=2, space="PSUM") as pst,
    ):
        scoresT = rt.tile([E, S], F32)
        pl = [psl.tile([E, 512], F32, tag="pl", name=f"pl{i}") for i in range(NSC)]
        HS2 = S // 2
        for k in range(KC):
            # two half-chunk tiles with separate tags: the WAR on slot reuse
            # releases per half, so the DMA stream runs ahead of the PE
            xlo = xs.tile([128, HS2], F32, tag="xkl", name="xlo", bufs=3)
            xhi = xs.tile([128, HS2], F32, tag="xkh", name="xhi")
            eng = nc.sync if k % 2 == 0 else nc.gpsimd
            oth = nc.gpsimd if k % 2 == 0 else nc.sync
            if k == 0:
                nc.sync.dma_start(xlo[:, :512], x_d[:128, :512])
                nc.gpsimd.dma_start(xlo[:, 512:], x_d[:128, 512:HS2])
                nc.sync.dma_start(xhi[:], x_d[:128, HS2:])
            else:
                eng.dma_start(xlo[:], x_d[k * 128:(k + 1) * 128, :HS2])
                oth.dma_start(xhi[:], x_d[k * 128:(k + 1) * 128, HS2:])
            # fp32r rounding copies for the FFN path
            nc.vector.tensor_copy(x_r[:, k * S:k * S + HS2], xlo[:])
            nc.vector.tensor_copy(x_r[:, k * S + HS2:(k + 1) * S], xhi[:])
            for sc in range(NSC):
                src_t = xlo if sc < 2 else xhi
                nc.tensor.matmul(
                    pl[sc][:],
                    rw[:, k * E:(k + 1) * E],
                    src_t[:, (sc % 2) * 512:(sc % 2 + 1) * 512],
                    start=(k == 0), stop=(k == KC - 1),
                )
        load_w(0)
        for sc in range(NSC):
            nc.scalar.activation(scoresT[:, sc * 512:(sc + 1) * 512], pl[sc][:],
                                 AF.Sigmoid)

        # transpose scores -> [s, (t e)] layout
        scores = rt.tile([128, NT * E], F32)
        for t in range(NT):
            pt = pst.tile([128, E], F32, tag="pt")
            nc.tensor.transpose(pt[:], scoresT[:, t * 128:(t + 1) * 128],
                                ident[:E, :E])
            nc.vector.tensor_copy(scores[:, t * E:(t + 1) * E], pt[:])

        # ---- routing math (all DVE), layout [128, (t=16, e=16)] ----
        sb = rt.tile([128, NT * E], F32)
        bias_exp = rt.tile([128, E], F32)
        nc.gpsimd.partition_broadcast(bias_exp[:], bias_sb[0:1, :])
        sbv = sb.rearrange("p (t e) -> p t e", t=NT)
        scv = scores.rearrange("p (t e) -> p t e", t=NT)
        nc.vector.tensor_add(
            sbv, scv, bias_exp[:, None, :].to_broadcast([128, NT, E]))

        # group top-2 sum over each group of 4: max over the 6 pairwise sums
        sbg = sb.rearrange("p (t g j) -> p t g j", t=NT, g=G)
        t2s = rt.tile([128, NT * G], F32)
        t2sv = t2s.rearrange("p (t g) -> p t g", t=NT)
        tmp = rt.tile([128, NT * G], F32)
        tmpv = tmp.rearrange("p (t g) -> p t g", t=NT)
        pairs = [(a, b) for a in range(EPG) for b in range(a + 1, EPG)]
        first = True
        for (a, b) in pairs:
            dst = t2sv if first else tmpv
            nc.vector.tensor_add(dst, sbg[:, :, :, a], sbg[:, :, :, b])
            if not first:
                nc.vector.tensor_max(t2sv, t2sv, tmpv)
            first = False

        # second-largest group score per token: max over pairwise mins
        m2 = rt.tile([128, NT], F32)
        m2t = rt.tile([128, NT], F32)
        gpairs = [(a, b) for a in range(G) for b in range(a + 1, G)]
        first = True
        for (a, b) in gpairs:
            dst = m2 if first else m2t
            nc.vector.tensor_tensor(dst[:], t2sv[:, :, a], t2sv[:, :, b], OP.min)
            if not first:
                nc.vector.tensor_max(m2[:], m2[:], m2t[:])
            first = False

        # penalty: -1e30 on experts whose group is not in the top 2
        pen = rt.tile([128, NT * G], F32)
        penv = pen.rearrange("p (t g) -> p t g", t=NT)
        nc.vector.tensor_tensor(
            penv, t2sv, m2[:, :, None].to_broadcast([128, NT, G]), OP.is_lt)
        nc.vector.tensor_scalar_mul(pen[:], pen[:], -1e30)

        sbm = rt.tile([128, NT * E], F32)
        sbmg = sbm.rearrange("p (t g j) -> p t g j", t=NT, g=G)
        nc.vector.tensor_add(
            sbmg, sbg, penv[:, :, :, None].to_broadcast([128, NT, G, EPG]))

        # 4th largest of the masked biased scores per token -> threshold
        m8 = rt.tile([128, NT * 8], F32)
        for t in range(NT):
            nc.vector.max(m8[:, t * 8:(t + 1) * 8], sbm[:, t * E:(t + 1) * E])
        v4 = m8.rearrange("p (t k) -> p t k", t=NT)[:, :, TOPK - 1]

        msk = rt.tile([128, NT * E], F32)
        mskv = msk.rearrange("p (t e) -> p t e", t=NT)
        sbmv = sbm.rearrange("p (t e) -> p t e", t=NT)
        nc.vector.tensor_tensor(
            mskv, sbmv, v4[:, :, None].to_broadcast([128, NT, E]), OP.is_ge)

        # weights: unbiased scores at selected positions, renormalized
        wm = rt.tile([128, NT * E], F32)
        nc.vector.tensor_mul(wm[:], scores[:], msk[:])
        ws = rt.tile([128, NT], F32)
        nc.vector.reduce_sum(ws[:], wm.rearrange("p (t e) -> p t e", t=NT),
                             axis=mybir.AxisListType.X)
        nc.vector.tensor_scalar_add(ws[:], ws[:], 1e-20)
        wr = rt.tile([128, NT], F32)
        nc.vector.reciprocal(wr[:], ws[:])
        combv = comb.rearrange("p (t e) -> p t e", t=NT)
        nc.vector.tensor_mul(
            combv, wm.rearrange("p (t e) -> p t e", t=NT),
            wr[:, :, None].to_broadcast([128, NT, E]))

    # ---------------- FFN ----------------
    # down-proj weight loads (needed only in the down pass; emitted here so
    # they don't delay the x/router DMAs)
    for src in range(2):
        nc.sync.dma_start(
            wd[src].rearrange("p (hc c) -> p hc c", hc=NHC),
            dw_d[src].rearrange("(hc p) c -> p hc c", p=128))
    nc.sync.dma_start(
        wd[2].rearrange("p (hc c) -> p hc c", hc=NHC),
        sdw_d.rearrange("(hc p) c -> p hc c", p=128))

    cp = tc.alloc_tile_pool(name="cp", bufs=1)
    with (
        tc.tile_pool(name="cb", bufs=1) as cbp,
        tc.tile_pool(name="hsb", bufs=2) as hsb,
        tc.tile_pool(name="psg", bufs=3, space="PSUM") as psg,
        tc.tile_pool(name="psu", bufs=3, space="PSUM") as psu,
    ):
        combT = None
        for src in range(3):
            if src not in w_tiles:
                load_w(src)
            wt = w_tiles.pop(src)

            for hc in range(NHC):
                h_sb = hsb.tile([128, S], F32, tag="h")
                for sc in range(NSC):
                    pg = psg.tile([128, 512], F32, tag="pg")
                    pu = psu.tile([128, 512], F32, tag="pu")
                    for k in range(KC):
                        nc.tensor.matmul(
                            pg[:],
                            wt[("g", hc)][:, k * 128:(k + 1) * 128],
                            x_r[:, k * S + sc * 512: k * S + (sc + 1) * 512],
                            start=(k == 0), stop=(k == KC - 1))
                    for k in range(KC):
                        nc.tensor.matmul(
                            pu[:],
                            wt[("u", hc)][:, k * 128:(k + 1) * 128],
                            x_r[:, k * S + sc * 512: k * S + (sc + 1) * 512],
                            start=(k == 0), stop=(k == KC - 1))
                    sl = slice(sc * 512, (sc + 1) * 512)
                    nc.scalar.activation(h_sb[:, sl], pg[:], AF.Silu)
                    if src == 2:
                        # shared expert: no combine scaling; write f32r directly
                        nc.vector.tensor_mul(hw[src][hc][:, sl], h_sb[:, sl],
                                             pu[:])
                    else:
                        nc.vector.tensor_mul(h_sb[:, sl], h_sb[:, sl], pu[:])

                if src == 0 and combT is None:
                    # emit combine transposes after the first expert's g/u
                    # matmuls so the PE isn't stalled on the routing DVE chain
                    combT = cp.tile([E, S], F32R)
                    with tc.tile_pool(name="psct", bufs=2,
                                      space="PSUM") as psc:
                        for t in range(NT):
                            pct = psc.tile([E, 128], F32, tag="pct")
                            nc.tensor.transpose(
                                pct[:], comb[:, t * E:(t + 1) * E], ident[:])
                            nc.vector.tensor_copy(
                                combT[:, t * 128:(t + 1) * 128], pct[:])

                if src < 2 and hc == 0:
                    # broadcast this core's combine row across partitions by
                    # multiplying with a column-replicated one-hot (PE)
                    cb_exp = cbp.tile([128, S], F32, tag="cb", name="cb_exp")
                    with tc.tile_pool(name="pse2", bufs=2,
                                      space="PSUM") as pse2p:
                        for sc in range(NSC):
                            pe2 = pse2p.tile([128, 512], F32, tag="pe2")
                            nc.tensor.matmul(
                                pe2[:], esel[:, src * 128:(src + 1) * 128],
                                combT[:, sc * 512:(sc + 1) * 512],
                                start=True, stop=True)
                            nc.vector.tensor_copy(
                                cb_exp[:, sc * 512:(sc + 1) * 512], pe2[:])
                    cb_cur = cb_exp

                if src < 2:
                    nc.vector.tensor_mul(hw[src][hc][:], h_sb[:], cb_cur[:])

    cp.release()
    wp.release()
    xr_pool.release()

    # ---------------- down projection ----------------
    with (
        tc.tile_pool(name="oso", bufs=2) as oso,
        tc.tile_pool(name="pso", bufs=4, space="PSUM") as pso,
    ):
        for cc in range(NCC):
            os_t = oso.tile([128, S], F32, tag="os")
            for sc in range(NSC):
                po = pso.tile([128, 512], F32, tag="po")
                idx = 0
                for src in range(3):
                    for hc in range(NHC):
                        nc.tensor.matmul(
                            po[:],
                            wd[src][:, hc * C + cc * 128: hc * C + (cc + 1) * 128],
                            hw[src][hc][:, sc * 512:(sc + 1) * 512],
                            start=(idx == 0), stop=(idx == 5))
                        idx += 1
                nc.vector.tensor_copy(os_t[:, sc * 512:(sc + 1) * 512], po[:])
                if cc == NCC - 1:
                    oeng = nc.sync if sc % 2 == 0 else nc.gpsimd
                    oeng.dma_start(
                        out_d[cc * 128:(cc + 1) * 128,
                              sc * 512:(sc + 1) * 512],
                        os_t[:, sc * 512:(sc + 1) * 512])
            if cc < NCC - 1:
                nc.sync.dma_start(out_d[cc * 128:(cc + 1) * 128, :], os_t[:])

    hw_pool.release()
    consts.release()


_NC_CACHE = {}


def _get_nc():
    if "nc" not in _NC_CACHE:
        _NC_CACHE["nc"] = build()
    return _NC_CACHE["nc"]


def make_in_maps(x, router_w, correction_bias, gate_w, up_w, down_w,
                 shared_gate_w, shared_up_w, shared_down_w):
    x = np.asarray(x, dtype=np.float32)
    xT = np.ascontiguousarray(x.reshape(S, C).T)                 # [C, S]
    rwT = np.asarray(router_w, dtype=np.float32).T               # [C, E]
    rw_pk = np.ascontiguousarray(
        rwT.reshape(KC, 128, E).transpose(1, 0, 2).reshape(128, KC * E))
    bias = np.asarray(correction_bias, dtype=np.float32).reshape(1, E)
    sgT = np.asarray(shared_gate_w, dtype=np.float32).T          # [C, HS]
    suT = np.asarray(shared_up_w, dtype=np.float32).T            # [C, HS]
    sdT = np.asarray(shared_down_w, dtype=np.float32).T          # [HS, C]
    gate_w = np.asarray(gate_w, dtype=np.float32)
    up_w = np.asarray(up_w, dtype=np.float32)
    down_w = np.asarray(down_w, dtype=np.float32)

    in_maps = []
    for c in range(NCORES):
        es = slice(c * EPC, (c + 1) * EPC)
        hs = slice(c * HSL, (c + 1) * HSL)
        esel = np.zeros((E, EPC * 128), np.float32)
        esel[c * EPC, 0:128] = 1.0
        esel[c * EPC + 1, 128:256] = 1.0
        in_maps.append({
            "xT": xT,
            "rw": rw_pk,
            "bias": bias,
            "esel": esel,
            "gw": _round_f32r(gate_w[es]),
            "uw": _round_f32r(up_w[es]),
            "dw": _round_f32r(down_w[es]),
            "sgw": _round_f32r(sgT[:, hs]),
            "suw": _round_f32r(suT[:, hs]),
            "sdw": _round_f32r(sdT[hs, :]),
        })
    return in_maps


def kernel(x, router_w, correction_bias, gate_w, up_w, down_w,
           shared_gate_w, shared_up_w, shared_down_w):
    in_maps = make_in_maps(x, router_w, correction_bias, gate_w, up_w, down_w,
                           shared_gate_w, shared_up_w, shared_down_w)
    nc = _get_nc()
    res = run_bass_kernel_spmd(nc, in_maps, list(range(NCORES)))
    acc = np.zeros((C, S), np.float64)
    for c in range(NCORES):
        acc += res.results[c]["out"].astype(np.float64)
    return np.ascontiguousarray(acc.T).astype(np.float32).reshape(B, T, C)



# revision 4
# speedup vs baseline: 1.9516x; 1.9516x over previous
"""MoE FFN (grouped sigmoid top-k routing + shared expert) on 8 TRN2 NeuronCores.

Strategy: expert-parallel with SPARSE token dispatch. Routing is computed on
the host (it determines the sharding itself — this harness's stand-in for the
"all-to-all token dispatch after routing" in the sharding hint): each core
gets 2 of 16 routed experts and receives only the tokens routed to them
(gathered + padded to a static capacity CAP per expert), plus 1/8 of the
shared expert (sharded along its hidden dim HS) over all tokens. All device
matmuls run in bf16 (rel-err budget 2e-2; bf16 lands ~2e-3), halving DMA and
SBUF versus fp32r at the same PE rate. Each core writes a dense shared-expert
partial [C, S] and its two experts' gathered outputs [C, CAP] (un-weighted);
the host applies the combine weights, scatter-adds, reduces over cores, and
transposes back.

Device work per core: 2*CAP*3*C*H (routed, ~1.1x the true top-4 FLOPs) +
3*S*C*HS/8 (shared) MACs ~ 2.4 G MACs -> ~64 us PE at 2.4 GHz bf16, vs the
dense-dispatch baseline's ~166 us.
"""

import numpy as np
import ml_dtypes

import concourse.bacc as bacc
import concourse.mybir as mybir
from concourse import tile
from concourse.bass_utils import run_bass_kernel_spmd

F32 = mybir.dt.float32
BF = mybir.dt.bfloat16
AF = mybir.ActivationFunctionType
BF_NP = ml_dtypes.bfloat16

# problem shapes (hardcoded; kernel.py must be self-contained)
B, T, C, H, HS = 2, 1024, 1024, 256, 2048
E, G, EPG = 16, 4, 4
TOPK, TOPK_GROUP = 4, 2
PER_GROUP_K = TOPK // TOPK_GROUP
NCORES = 8
S = B * T                  # 2048 tokens
EPC = E // NCORES          # 2 experts per core
HSL = HS // NCORES         # 256 shared-hidden rows per core
KC = C // 128              # 8 contraction chunks
NHC = H // 128             # 2 h chunks (same for HSL)
NSC = S // 512             # 4 moving (token) chunks of 512
NCC = C // 128             # 8 output-row chunks
CAP = 576                  # per-expert token capacity (mean load is 512)
NCH = 2                    # moving chunks per expert
CH = CAP // NCH            # 288 tokens per chunk (psum bank holds 512 fp32)


def build():
    nc = bacc.Bacc(
        "TRN2",
        target_bir_lowering=False,
        debug=False,
        enable_asserts=True,
        num_devices=NCORES,
    )
    # ---- DRAM I/O (per core), all bf16, pre-packed [128, ...] on host ----
    xt_d = nc.declare_dram_parameter("xt", [128, KC * S], BF, isOutput=False)
    xg_d = nc.declare_dram_parameter("xg", [128, KC * EPC * CAP], BF,
                                     isOutput=False)
    wgu_d = nc.declare_dram_parameter("wgu", [128, EPC * 2 * KC * H], BF,
                                      isOutput=False)
    wdn_d = nc.declare_dram_parameter("wdn", [128, EPC * NHC * C], BF,
                                      isOutput=False)
    wsgu_d = nc.declare_dram_parameter("wsgu", [128, 2 * KC * HSL], BF,
                                       isOutput=False)
    wsdn_d = nc.declare_dram_parameter("wsdn", [128, NHC * C], BF,
                                       isOutput=False)
    outS_d = nc.declare_dram_parameter("outS", [C, S], BF, isOutput=True)
    outR_d = nc.declare_dram_parameter("outR", [C, EPC * CAP], BF,
                                       isOutput=True)

    with tile.TileContext(nc) as tc:
        _emit(nc, tc, xt_d, xg_d, wgu_d, wdn_d, wsgu_d, wsdn_d, outS_d, outR_d)
    nc.finalize()
    return nc


def _emit(nc, tc, xt_d, xg_d, wgu_d, wdn_d, wsgu_d, wsdn_d, outS_d, outR_d):
    res = tc.alloc_tile_pool(name="res", bufs=1)
    xg = res.tile([128, KC * EPC * CAP], BF)
    wgu = res.tile([128, EPC * 2 * KC * H], BF)
    wdn = res.tile([128, EPC * NHC * C], BF)
    xt = res.tile([128, KC * S], BF)
    wsgu = res.tile([128, 2 * KC * HSL], BF)
    wsdn = res.tile([128, NHC * C], BF)
    hS = res.tile([128, NHC * S], BF)
    hR = [res.tile([128, NHC * CAP], BF, name=f"hR{j}") for j in range(EPC)]

    xgv = xg.rearrange("p (k j t) -> p k j t", k=KC, j=EPC)
    wguv = wgu.rearrange("p (j r k h) -> p j r k h", j=EPC, r=2, k=KC)
    wdnv = wdn.rearrange("p (j hk c) -> p j hk c", j=EPC, hk=NHC)
    xtv = xt.rearrange("p (k s) -> p k s", k=KC)
    wsguv = wsgu.rearrange("p (r k h) -> p r k h", r=2, k=KC)
    wsdnv = wsdn.rearrange("p (hk c) -> p hk c", hk=NHC)
    hSv = hS.rearrange("p (hk s) -> p hk s", hk=NHC)
    hRv = [t.rearrange("p (hk t) -> p hk t", hk=NHC) for t in hR]
    xg_dv = xg_d.rearrange("p (k j t) -> p k j t", k=KC, j=EPC)
    wgu_dv = wgu_d.rearrange("p (j r k h) -> p j r k h", j=EPC, r=2, k=KC)
    xt_dv = xt_d.rearrange("p (k s) -> p k s", k=KC)

    # ---- DMA schedule (2 queues), in order of first use ----
    # q_sync: xg per-k chunks (phase 1 consumes them incrementally), then
    # first xt half. q_gpsimd: routed weights, then second xt half + shared.
    for k in range(KC):
        nc.sync.dma_start(xgv[:, k], xg_dv[:, k])
    for j in range(EPC):
        for r in range(2):
            nc.gpsimd.dma_start(wguv[:, j, r], wgu_dv[:, j, r])
    nc.gpsimd.dma_start(wdn[:], wdn_d[:])
    for k in range(KC):
        eng = nc.sync if k < KC // 2 else nc.gpsimd
        eng.dma_start(xtv[:, k], xt_dv[:, k])
    nc.gpsimd.dma_start(wsgu[:], wsgu_d[:])
    nc.gpsimd.dma_start(wsdn[:], wsdn_d[:])

    # ---------------- phase 1: routed gate/up -> hR ----------------
    with (
        tc.tile_pool(name="p1g", bufs=2, space="PSUM") as p1g,
        tc.tile_pool(name="p1u", bufs=2, space="PSUM") as p1u,
        tc.tile_pool(name="s1", bufs=3) as s1,
    ):
        for j in range(EPC):
            for hc in range(NHC):
                for ch in range(NCH):
                    pg = p1g.tile([128, CH], F32, tag="pg")
                    pu = p1u.tile([128, CH], F32, tag="pu")
                    mv = [xgv[:, k, j, ch * CH:(ch + 1) * CH]
                          for k in range(KC)]
                    hsl = slice(hc * 128, (hc + 1) * 128)
                    for k in range(KC):
                        nc.tensor.matmul(pg[:], wguv[:, j, 0, k, hsl], mv[k],
                                         start=(k == 0), stop=(k == KC - 1))
                    for k in range(KC):
                        nc.tensor.matmul(pu[:], wguv[:, j, 1, k, hsl], mv[k],
                                         start=(k == 0), stop=(k == KC - 1))
                    tmp = s1.tile([128, CH], BF, tag="t")
                    nc.scalar.activation(tmp[:], pg[:], AF.Silu)
                    nc.vector.tensor_mul(
                        hRv[j][:, hc, ch * CH:(ch + 1) * CH], tmp[:], pu[:])

    # ---------------- phase 2: routed down -> outR ----------------
    with (
        tc.tile_pool(name="p2", bufs=4, space="PSUM") as p2,
        tc.tile_pool(name="s2", bufs=2) as s2,
    ):
        for cc in range(NCC):
            orr = s2.tile([128, EPC * CAP], BF, tag="or")
            cs = slice(cc * 128, (cc + 1) * 128)
            for j in range(EPC):
                for ch in range(NCH):
                    po = p2.tile([128, CH], F32, tag="po")
                    for hk in range(NHC):
                        nc.tensor.matmul(
                            po[:], wdnv[:, j, hk, cs],
                            hRv[j][:, hk, ch * CH:(ch + 1) * CH],
                            start=(hk == 0), stop=(hk == NHC - 1))
                    dst = orr[:, j * CAP + ch * CH: j * CAP + (ch + 1) * CH]
                    if (j * NCH + ch) % 2 == 0:
                        nc.scalar.copy(dst, po[:])
                    else:
                        nc.vector.tensor_copy(dst, po[:])
            nc.sync.dma_start(outR_d[cs, :], orr[:])

    # ---------------- phase 3: shared gate/up -> hS ----------------
    with (
        tc.tile_pool(name="p3g", bufs=2, space="PSUM") as p3g,
        tc.tile_pool(name="p3u", bufs=2, space="PSUM") as p3u,
        tc.tile_pool(name="s3", bufs=3) as s3,
    ):
        for hc in range(NHC):
            for sc in range(NSC):
                pg = p3g.tile([128, 512], F32, tag="pg")
                pu = p3u.tile([128, 512], F32, tag="pu")
                hsl = slice(hc * 128, (hc + 1) * 128)
                ss = slice(sc * 512, (sc + 1) * 512)
                for k in range(KC):
                    nc.tensor.matmul(pg[:], wsguv[:, 0, k, hsl],
                                     xtv[:, k, ss],
                                     start=(k == 0), stop=(k == KC - 1))
                for k in range(KC):
                    nc.tensor.matmul(pu[:], wsguv[:, 1, k, hsl],
                                     xtv[:, k, ss],
                                     start=(k == 0), stop=(k == KC - 1))
                tmp = s3.tile([128, 512], BF, tag="t")
                nc.scalar.activation(tmp[:], pg[:], AF.Silu)
                nc.vector.tensor_mul(hSv[:, hc, ss], tmp[:], pu[:])

    # ---------------- phase 4: shared down -> outS ----------------
    with (
        tc.tile_pool(name="p4", bufs=4, space="PSUM") as p4,
        tc.tile_pool(name="s4", bufs=2) as s4,
    ):
        for cc in range(NCC):
            osr = s4.tile([128, S], BF, tag="os")
            cs = slice(cc * 128, (cc + 1) * 128)
            for sc in range(NSC):
                po = p4.tile([128, 512], F32, tag="po")
                ss = slice(sc * 512, (sc + 1) * 512)
                for hk in range(NHC):
                    nc.tensor.matmul(po[:], wsdnv[:, hk, cs], hSv[:, hk, ss],
                                     start=(hk == 0), stop=(hk == NHC - 1))
                if sc % 2 == 0:
                    nc.scalar.copy(osr[:, ss], po[:])
                else:
                    nc.vector.tensor_copy(osr[:, ss], po[:])
            nc.sync.dma_start(outS_d[cs, :], osr[:])

    res.release()


_NC_CACHE = {}


def _get_nc():
    if "nc" not in _NC_CACHE:
        _NC_CACHE["nc"] = build()
    return _NC_CACHE["nc"]


def _route_host(xf, router_w, correction_bias):
    """Replicates reference._route in float64 numpy (stable argsort matches
    jax.lax.top_k's lower-index-wins tie-breaking)."""
    x64 = xf.astype(np.float64)
    logits = x64 @ router_w.astype(np.float64).T           # [S, E]
    scores = 1.0 / (1.0 + np.exp(-logits))
    sb = scores + correction_bias.astype(np.float64)
    n = sb.shape[0]
    sbg = sb.reshape(n, G, EPG)
    grp_top = -np.sort(-sbg, axis=-1)[:, :, :PER_GROUP_K]
    group_scores = grp_top.sum(axis=-1)                    # [S, G]
    gidx = np.argsort(-group_scores, kind="stable", axis=-1)[:, :TOPK_GROUP]
    gmask = np.zeros((n, G))
    np.put_along_axis(gmask, gidx, 1.0, axis=-1)
    smask = np.repeat(gmask, EPG, axis=1)
    masked = np.where(smask > 0, sb, -np.inf)
    tk = np.argsort(-masked, kind="stable", axis=-1)[:, :TOPK]   # [S, K]
    wv = np.take_along_axis(scores, tk, axis=1)
    wv = wv / (wv.sum(axis=-1, keepdims=True) + 1e-20)
    return tk, wv


def _expert_token_lists(tk, wv):
    """Per expert: (token idx ascending, combine weights). Overflow beyond
    CAP (statistically ~never at CAP=576 for mean load 512) drops the
    lowest-weight tokens."""
    out = []
    for e in range(E):
        tok, slot = np.nonzero(tk == e)
        w = wv[tok, slot]
        if len(tok) > CAP:
            keep = np.sort(np.argsort(-w)[:CAP])
            tok, w = tok[keep], w[keep]
        out.append((tok, w))
    return out


def _pack_contract(a):
    """[C_like, F] -> [128, (kc F)] with row index c = k*128 + p."""
    ck, f = a.shape
    kc = ck // 128
    return np.ascontiguousarray(
        a.reshape(kc, 128, f).transpose(1, 0, 2).reshape(128, kc * f))


def make_in_maps(x, router_w, correction_bias, gate_w, up_w, down_w,
                 shared_gate_w, shared_up_w, shared_down_w):
    x = np.asarray(x, dtype=np.float32)
    xf = x.reshape(S, C)
    tk, wv = _route_host(xf, np.asarray(router_w, np.float32),
                         np.asarray(correction_bias, np.float32))
    experts = _expert_token_lists(tk, wv)

    xT_bf = xf.T.astype(BF_NP)                              # [C, S]
    xt_pack = _pack_contract(xT_bf)                         # [128, KC*S]

    gate_w = np.asarray(gate_w, np.float32)
    up_w = np.asarray(up_w, np.float32)
    down_w = np.asarray(down_w, np.float32)
    sgT = np.asarray(shared_gate_w, np.float32).T           # [C, HS]
    suT = np.asarray(shared_up_w, np.float32).T             # [C, HS]
    sdT = np.asarray(shared_down_w, np.float32).T           # [HS, C]

    in_maps = []
    for c in range(NCORES):
        es = [c * EPC + j for j in range(EPC)]
        hs = slice(c * HSL, (c + 1) * HSL)

        # gathered tokens [128, (k j t)]
        xg = np.zeros((128, KC, EPC, CAP), BF_NP)
        for j, e in enumerate(es):
            tok, _w = experts[e]
            xsel = xf[tok].T.astype(BF_NP)                  # [C, n]
            xg[:, :, j, :len(tok)] = (
                xsel.reshape(KC, 128, len(tok)).transpose(1, 0, 2))
        # routed gate/up [128, (j r k h)]
        wgu = np.stack(
            [np.stack([_pack_contract(gate_w[e].astype(BF_NP)),
                       _pack_contract(up_w[e].astype(BF_NP))], 1)
             for e in es], 1)                               # [128, j, 2, KC*H]
        # routed down [128, (j hk c)]
        wdn = np.stack([_pack_contract(down_w[e].astype(BF_NP))
                        for e in es], 1)
        wsgu = np.stack([_pack_contract(sgT[:, hs].astype(BF_NP)),
                         _pack_contract(suT[:, hs].astype(BF_NP))], 1)
        wsdn = _pack_contract(sdT[hs, :].astype(BF_NP))

        in_maps.append({
            "xt": xt_pack,
            "xg": np.ascontiguousarray(xg.reshape(128, KC * EPC * CAP)),
            "wgu": np.ascontiguousarray(wgu.reshape(128, -1)),
            "wdn": np.ascontiguousarray(wdn.reshape(128, -1)),
            "wsgu": np.ascontiguousarray(wsgu.reshape(128, -1)),
            "wsdn": np.ascontiguousarray(wsdn),
        })
    return in_maps, experts


def postprocess(results, experts):
    accT = np.zeros((C, S), np.float64)
    for c in range(NCORES):
        accT += np.asarray(results[c]["outS"]).astype(np.float64)
        outR = np.asarray(results[c]["outR"]).astype(np.float64)
        for j in range(EPC):
            tok, w = experts[c * EPC + j]
            accT[:, tok] += outR[:, j * CAP: j * CAP + len(tok)] * w[None, :]
    return np.ascontiguousarray(accT.T).astype(np.float32).reshape(B, T, C)


def kernel(x, router_w, correction_bias, gate_w, up_w, down_w,
           shared_gate_w, shared_up_w, shared_down_w):
    in_maps, experts = make_in_maps(
        x, router_w, correction_bias, gate_w, up_w, down_w,
        shared_gate_w, shared_up_w, shared_down_w)
    nc = _get_nc()
    res = run_bass_kernel_spmd(nc, in_maps, list(range(NCORES)))
    return postprocess(res.results, experts)


# revision 15
# speedup vs baseline: 2.3462x; 1.2022x over previous
"""MoE FFN (grouped sigmoid top-k routing + shared expert) on 8 TRN2 NeuronCores.

Strategy: expert-parallel with SPARSE token dispatch. Routing is computed on
the host (it determines the sharding itself — this harness's stand-in for the
"all-to-all token dispatch after routing" in the sharding hint): each core
gets 2 of 16 routed experts and receives only the tokens routed to them
(gathered + padded to a static capacity CAP per expert), plus 1/8 of the
shared expert (sharded along its hidden dim HS) over all tokens. All device
matmuls run in bf16 (rel-err budget 2e-2; bf16 lands ~2e-3), halving DMA and
SBUF versus fp32r at the same PE rate. Each core writes a dense shared-expert
partial [C, S] and its two experts' gathered outputs [C, CAP] (un-weighted);
the host applies the combine weights, scatter-adds, reduces over cores, and
transposes back.

Device work per core: 2*CAP*3*C*H (routed, ~1.1x the true top-4 FLOPs) +
3*S*C*HS/8 (shared) MACs ~ 2.4 G MACs -> ~64 us PE at 2.4 GHz bf16, vs the
dense-dispatch baseline's ~166 us.
"""

import numpy as np
import ml_dtypes

import concourse.bacc as bacc
import concourse.mybir as mybir
from concourse import tile
from concourse.bass_utils import run_bass_kernel_spmd

F32 = mybir.dt.float32
BF = mybir.dt.bfloat16
AF = mybir.ActivationFunctionType
BF_NP = ml_dtypes.bfloat16

# problem shapes (hardcoded; kernel.py must be self-contained)
B, T, C, H, HS = 2, 1024, 1024, 256, 2048
E, G, EPG = 16, 4, 4
TOPK, TOPK_GROUP = 4, 2
PER_GROUP_K = TOPK // TOPK_GROUP
NCORES = 8
S = B * T                  # 2048 tokens
EPC = E // NCORES          # 2 experts per core
HSL = HS // NCORES         # 256 shared-hidden rows per core
KC = C // 128              # 8 contraction chunks
NHC = H // 128             # 2 h chunks (same for HSL)
NSC = S // 512             # 4 moving (token) chunks of 512
NCC = C // 128             # 8 output-row chunks
CAP = 576                  # per-expert token capacity (mean load is 512)
NCH = 2                    # moving chunks per expert
CH = CAP // NCH            # 288 tokens per chunk (psum bank holds 512 fp32)


def build():
    nc = bacc.Bacc(
        "TRN2",
        target_bir_lowering=False,
        debug=False,
        enable_asserts=True,
        num_devices=NCORES,
    )
    # ---- DRAM I/O (per core), all bf16, pre-packed [128, ...] on host ----
    xt_d = nc.declare_dram_parameter("xt", [128, KC * S], BF, isOutput=False)
    xg_d = nc.declare_dram_parameter("xg", [128, KC * EPC * CAP], BF,
                                     isOutput=False)
    wgu_d = nc.declare_dram_parameter("wgu", [128, EPC * 2 * KC * H], BF,
                                      isOutput=False)
    wdn_d = nc.declare_dram_parameter("wdn", [128, EPC * NHC * C], BF,
                                      isOutput=False)
    wsgu_d = nc.declare_dram_parameter("wsgu", [128, 2 * KC * HSL], BF,
                                       isOutput=False)
    wsdn_d = nc.declare_dram_parameter("wsdn", [128, NHC * C], BF,
                                       isOutput=False)
    outS_d = nc.declare_dram_parameter("outS", [C, S], BF, isOutput=True)
    outR_d = nc.declare_dram_parameter("outR", [C, EPC * CAP], BF,
                                       isOutput=True)

    with tile.TileContext(nc) as tc:
        _emit(nc, tc, xt_d, xg_d, wgu_d, wdn_d, wsgu_d, wsdn_d, outS_d, outR_d)
    nc.finalize()
    return nc


def _emit(nc, tc, xt_d, xg_d, wgu_d, wdn_d, wsgu_d, wsdn_d, outS_d, outR_d):
    res = tc.alloc_tile_pool(name="res", bufs=1)
    xg = res.tile([128, KC * EPC * CAP], BF)
    wgu = res.tile([128, EPC * 2 * KC * H], BF)
    wdn = res.tile([128, EPC * NHC * C], BF)
    xt = res.tile([128, KC * S], BF)
    wsgu = res.tile([128, 2 * KC * HSL], BF)
    wsdn = res.tile([128, NHC * C], BF)
    hS = res.tile([128, NHC * S], BF)
    hR = [res.tile([128, NHC * CAP], BF, name=f"hR{j}") for j in range(EPC)]

    xgv = xg.rearrange("p (k j t) -> p k j t", k=KC, j=EPC)
    wguv = wgu.rearrange("p (j r k h) -> p j r k h", j=EPC, r=2, k=KC)
    wdnv = wdn.rearrange("p (j hk c) -> p j hk c", j=EPC, hk=NHC)
    xtv = xt.rearrange("p (k s) -> p k s", k=KC)
    wsguv = wsgu.rearrange("p (r k h) -> p r k h", r=2, k=KC)
    wsdnv = wsdn.rearrange("p (hk c) -> p hk c", hk=NHC)
    hSv = hS.rearrange("p (hk s) -> p hk s", hk=NHC)
    hRv = [t.rearrange("p (hk t) -> p hk t", hk=NHC) for t in hR]
    xg_dv = xg_d.rearrange("p (k j t) -> p k j t", k=KC, j=EPC)
    wgu_dv = wgu_d.rearrange("p (j r k h) -> p j r k h", j=EPC, r=2, k=KC)
    xt_dv = xt_d.rearrange("p (k s) -> p k s", k=KC)

    # ---- DMA schedule. Transfers serialize per issuing queue (~330 GB/s
    # each in the model): sync+pool carry the bulk; vector/scalar queues
    # (idle until the first psum results, ~4.5us in) bootstrap the routed
    # gate/up weights so phase 1's working set lands within ~2us.
    # phase-1 expert-0 working set first: gate weights + tokens, chunked to
    # match PE consumption order
    wgu_fv = wgu_d.rearrange("p (j r k h) -> p j r k h", j=EPC, r=2, k=KC)
    nc.sync.dma_start(wguv[:, 0, 0, :KC // 2], wgu_fv[:, 0, 0, :KC // 2])
    for k in range(KC):
        nc.gpsimd.dma_start(xgv[:, k, 0], xg_dv[:, k, 0])
    nc.sync.dma_start(wguv[:, 0, 0, KC // 2:], wgu_fv[:, 0, 0, KC // 2:])
    nc.sync.dma_start(wguv[:, 0, 1, :KC // 2], wgu_fv[:, 0, 1, :KC // 2])
    nc.sync.dma_start(wguv[:, 0, 1, KC // 2:], wgu_fv[:, 0, 1, KC // 2:])
    nc.scalar.dma_start(wguv[:, 1, 0], wgu_dv[:, 1, 0])
    nc.scalar.dma_start(wguv[:, 1, 1], wgu_dv[:, 1, 1])
    for k in range(KC):
        (nc.sync if k % 2 == 0 else nc.gpsimd).dma_start(
            xgv[:, k, 1], xg_dv[:, k, 1])
    nc.sync.dma_start(wsgu[:], wsgu_d[:])
    nc.gpsimd.dma_start(wdn[:], wdn_d[:])
    for k in range(KC):
        eng = nc.sync if k < KC // 2 else nc.gpsimd
        eng.dma_start(xtv[:, k], xt_dv[:, k])
    nc.gpsimd.dma_start(wsdn[:], wsdn_d[:])

    # Persistent PSUM pools spanning all phases (all 8 banks; no per-phase
    # scope-close barriers): pg/pu double-buffered for gate/up, po 4-deep
    # for the down projections.
    ppg = tc.alloc_tile_pool(name="ppg", bufs=2, space="PSUM")
    ppu = tc.alloc_tile_pool(name="ppu", bufs=2, space="PSUM")
    ppo = tc.alloc_tile_pool(name="ppo", bufs=4, space="PSUM")
    stmp = tc.alloc_tile_pool(name="stmp", bufs=3)
    sout = tc.alloc_tile_pool(name="sout", bufs=2)

    # ---------------- phase 1: routed gate/up -> hR ----------------
    for j in range(EPC):
        for hc in range(NHC):
            for ch in range(NCH):
                pg = ppg.tile([128, 512], F32, tag="pg")
                pu = ppu.tile([128, 512], F32, tag="pu")
                mv = [xgv[:, k, j, ch * CH:(ch + 1) * CH]
                      for k in range(KC)]
                hsl = slice(hc * 128, (hc + 1) * 128)
                for k in range(KC):
                    nc.tensor.matmul(pg[:, :CH], wguv[:, j, 0, k, hsl], mv[k],
                                     start=(k == 0), stop=(k == KC - 1))
                for k in range(KC):
                    nc.tensor.matmul(pu[:, :CH], wguv[:, j, 1, k, hsl], mv[k],
                                     start=(k == 0), stop=(k == KC - 1))
                tmp = stmp.tile([128, CH], BF, tag="t1")
                nc.scalar.activation(tmp[:], pg[:, :CH], AF.Silu)
                nc.vector.tensor_mul(
                    hRv[j][:, hc, ch * CH:(ch + 1) * CH], tmp[:], pu[:, :CH])

    # ---------------- phase 2: routed down -> outR ----------------
    for cc in range(NCC):
        orr = sout.tile([128, EPC * CAP], BF, tag="or")
        cs = slice(cc * 128, (cc + 1) * 128)
        for j in range(EPC):
            for ch in range(NCH):
                po = ppo.tile([128, 512], F32, tag="po")
                for hk in range(NHC):
                    nc.tensor.matmul(
                        po[:, :CH], wdnv[:, j, hk, cs],
                        hRv[j][:, hk, ch * CH:(ch + 1) * CH],
                        start=(hk == 0), stop=(hk == NHC - 1))
                dst = orr[:, j * CAP + ch * CH: j * CAP + (ch + 1) * CH]
                if (j * NCH + ch) % 2 == 0:
                    nc.scalar.copy(dst, po[:, :CH])
                else:
                    nc.vector.tensor_copy(dst, po[:, :CH])
        eng = nc.sync if cc % 2 == 0 else nc.gpsimd
        eng.dma_start(outR_d[cs, :], orr[:])

    # ---------------- phase 3: shared gate/up -> hS ----------------
    for hc in range(NHC):
        for sc in range(NSC):
            pg = ppg.tile([128, 512], F32, tag="pg")
            pu = ppu.tile([128, 512], F32, tag="pu")
            hsl = slice(hc * 128, (hc + 1) * 128)
            ss = slice(sc * 512, (sc + 1) * 512)
            for k in range(KC):
                nc.tensor.matmul(pg[:], wsguv[:, 0, k, hsl], xtv[:, k, ss],
                                 start=(k == 0), stop=(k == KC - 1))
            for k in range(KC):
                nc.tensor.matmul(pu[:], wsguv[:, 1, k, hsl], xtv[:, k, ss],
                                 start=(k == 0), stop=(k == KC - 1))
            tmp = stmp.tile([128, 512], BF, tag="t3")
            nc.scalar.activation(tmp[:], pg[:], AF.Silu)
            nc.vector.tensor_mul(hSv[:, hc, ss], tmp[:], pu[:])

    # ---------------- phase 4: shared down -> outS ----------------
    for cc in range(NCC):
        osr = sout.tile([128, S], BF, tag="os")
        cs = slice(cc * 128, (cc + 1) * 128)
        for sc in range(NSC):
            po = ppo.tile([128, 512], F32, tag="po")
            ss = slice(sc * 512, (sc + 1) * 512)
            for hk in range(NHC):
                nc.tensor.matmul(po[:], wsdnv[:, hk, cs], hSv[:, hk, ss],
                                 start=(hk == 0), stop=(hk == NHC - 1))
            if sc % 2 == 0:
                nc.scalar.copy(osr[:, ss], po[:])
            else:
                nc.vector.tensor_copy(osr[:, ss], po[:])
            # stream the output row out as its chunks complete; the final
            # row goes out per-chunk on alternating queues so the tail
            # after the last matmul stays short
            if cc < NCC - 1:
                if sc == 1:
                    nc.sync.dma_start(outS_d[cs, :1024], osr[:, :1024])
                elif sc == 3:
                    nc.gpsimd.dma_start(outS_d[cs, 1024:], osr[:, 1024:])
            else:
                eng = nc.sync if sc % 2 == 0 else nc.gpsimd
                eng.dma_start(outS_d[cs, ss], osr[:, ss])

    sout.release()
    stmp.release()
    ppo.release()
    ppu.release()
    ppg.release()
    res.release()


_NC_CACHE = {}


def _get_nc():
    if "nc" not in _NC_CACHE:
        _NC_CACHE["nc"] = build()
    return _NC_CACHE["nc"]


def _route_host(xf, router_w, correction_bias):
    """Replicates reference._route in float64 numpy (stable argsort matches
    jax.lax.top_k's lower-index-wins tie-breaking)."""
    x64 = xf.astype(np.float64)
    logits = x64 @ router_w.astype(np.float64).T           # [S, E]
    scores = 1.0 / (1.0 + np.exp(-logits))
    sb = scores + correction_bias.astype(np.float64)
    n = sb.shape[0]
    sbg = sb.reshape(n, G, EPG)
    grp_top = -np.sort(-sbg, axis=-1)[:, :, :PER_GROUP_K]
    group_scores = grp_top.sum(axis=-1)                    # [S, G]
    gidx = np.argsort(-group_scores, kind="stable", axis=-1)[:, :TOPK_GROUP]
    gmask = np.zeros((n, G))
    np.put_along_axis(gmask, gidx, 1.0, axis=-1)
    smask = np.repeat(gmask, EPG, axis=1)
    masked = np.where(smask > 0, sb, -np.inf)
    tk = np.argsort(-masked, kind="stable", axis=-1)[:, :TOPK]   # [S, K]
    wv = np.take_along_axis(scores, tk, axis=1)
    wv = wv / (wv.sum(axis=-1, keepdims=True) + 1e-20)
    return tk, wv


def _expert_token_lists(tk, wv):
    """Per expert: (token idx ascending, combine weights). Overflow beyond
    CAP (statistically ~never at CAP=576 for mean load 512) drops the
    lowest-weight tokens."""
    out = []
    for e in range(E):
        tok, slot = np.nonzero(tk == e)
        w = wv[tok, slot]
        if len(tok) > CAP:
            keep = np.sort(np.argsort(-w)[:CAP])
            tok, w = tok[keep], w[keep]
        out.append((tok, w))
    return out


def _pack_contract(a):
    """[C_like, F] -> [128, (kc F)] with row index c = k*128 + p."""
    ck, f = a.shape
    kc = ck // 128
    return np.ascontiguousarray(
        a.reshape(kc, 128, f).transpose(1, 0, 2).reshape(128, kc * f))


def make_in_maps(x, router_w, correction_bias, gate_w, up_w, down_w,
                 shared_gate_w, shared_up_w, shared_down_w):
    x = np.asarray(x, dtype=np.float32)
    xf = x.reshape(S, C)
    tk, wv = _route_host(xf, np.asarray(router_w, np.float32),
                         np.asarray(correction_bias, np.float32))
    experts = _expert_token_lists(tk, wv)

    xT_bf = xf.T.astype(BF_NP)                              # [C, S]
    xt_pack = _pack_contract(xT_bf)                         # [128, KC*S]

    gate_w = np.asarray(gate_w, np.float32)
    up_w = np.asarray(up_w, np.float32)
    down_w = np.asarray(down_w, np.float32)
    sgT = np.asarray(shared_gate_w, np.float32).T           # [C, HS]
    suT = np.asarray(shared_up_w, np.float32).T             # [C, HS]
    sdT = np.asarray(shared_down_w, np.float32).T           # [HS, C]

    in_maps = []
    for c in range(NCORES):
        es = [c * EPC + j for j in range(EPC)]
        hs = slice(c * HSL, (c + 1) * HSL)

        # gathered tokens [128, (k j t)]
        xg = np.zeros((128, KC, EPC, CAP), BF_NP)
        for j, e in enumerate(es):
            tok, _w = experts[e]
            xsel = xf[tok].T.astype(BF_NP)                  # [C, n]
            xg[:, :, j, :len(tok)] = (
                xsel.reshape(KC, 128, len(tok)).transpose(1, 0, 2))
        # routed gate/up [128, (j r k h)]
        wgu = np.stack(
            [np.stack([_pack_contract(gate_w[e].astype(BF_NP)),
                       _pack_contract(up_w[e].astype(BF_NP))], 1)
             for e in es], 1)                               # [128, j, 2, KC*H]
        # routed down [128, (j hk c)]
        wdn = np.stack([_pack_contract(down_w[e].astype(BF_NP))
                        for e in es], 1)
        wsgu = np.stack([_pack_contract(sgT[:, hs].astype(BF_NP)),
                         _pack_contract(suT[:, hs].astype(BF_NP))], 1)
        wsdn = _pack_contract(sdT[hs, :].astype(BF_NP))

        in_maps.append({
            "xt": xt_pack,
            "xg": np.ascontiguousarray(xg.reshape(128, KC * EPC * CAP)),
            "wgu": np.ascontiguousarray(wgu.reshape(128, -1)),
            "wdn": np.ascontiguousarray(wdn.reshape(128, -1)),
            "wsgu": np.ascontiguousarray(wsgu.reshape(128, -1)),
            "wsdn": np.ascontiguousarray(wsdn),
        })
    return in_maps, experts


def postprocess(results, experts):
    accT = np.zeros((C, S), np.float64)
    for c in range(NCORES):
        accT += np.asarray(results[c]["outS"]).astype(np.float64)
        outR = np.asarray(results[c]["outR"]).astype(np.float64)
        for j in range(EPC):
            tok, w = experts[c * EPC + j]
            accT[:, tok] += outR[:, j * CAP: j * CAP + len(tok)] * w[None, :]
    return np.ascontiguousarray(accT.T).astype(np.float32).reshape(B, T, C)


def kernel(x, router_w, correction_bias, gate_w, up_w, down_w,
           shared_gate_w, shared_up_w, shared_down_w):
    in_maps, experts = make_in_maps(
        x, router_w, correction_bias, gate_w, up_w, down_w,
        shared_gate_w, shared_up_w, shared_down_w)
    nc = _get_nc()
    res = run_bass_kernel_spmd(nc, in_maps, list(range(NCORES)))
    return postprocess(res.results, experts)


# revision 21
# speedup vs baseline: 2.3936x; 1.0202x over previous
"""MoE FFN (grouped sigmoid top-k routing + shared expert) on 8 TRN2 NeuronCores.

Strategy: expert-parallel with SPARSE token dispatch. Routing is computed on
the host (it determines the sharding itself — this harness's stand-in for the
"all-to-all token dispatch after routing" in the sharding hint): each core
gets 2 of 16 routed experts and receives only the tokens routed to them
(gathered + padded to a per-call capacity CAP per expert), plus 1/8 of the
shared expert (sharded along its hidden dim HS) over all tokens. All device
matmuls run in bf16 (rel-err budget 2e-2; bf16 lands ~5e-3), halving DMA and
SBUF versus fp32r at the same PE rate. Each core writes a dense shared-expert
partial [C, S] and its two experts' gathered outputs [C, CAP] (un-weighted);
the host applies the combine weights, scatter-adds, reduces over cores, and
transposes back.

CAP is sized to the call's actual max per-expert load (rounded up to even),
so no tokens are ever dropped; the compiled kernel is cached per CAP.
Device work per core: 2*CAP*3*C*H (routed) + 3*S*C*HS/8 (shared) MACs
~ 2.4 G MACs -> ~63 us PE at 2.4 GHz bf16, vs the dense-dispatch baseline's
~166 us.
"""

import numpy as np
import ml_dtypes

import concourse.bacc as bacc
import concourse.mybir as mybir
from concourse import tile
from concourse.bass_utils import run_bass_kernel_spmd

F32 = mybir.dt.float32
BF = mybir.dt.bfloat16
AF = mybir.ActivationFunctionType
BF_NP = ml_dtypes.bfloat16

# problem shapes (hardcoded; kernel.py must be self-contained)
B, T, C, H, HS = 2, 1024, 1024, 256, 2048
E, G, EPG = 16, 4, 4
TOPK, TOPK_GROUP = 4, 2
PER_GROUP_K = TOPK // TOPK_GROUP
NCORES = 8
S = B * T                  # 2048 tokens
EPC = E // NCORES          # 2 experts per core
HSL = HS // NCORES         # 256 shared-hidden rows per core
KC = C // 128              # 8 contraction chunks
NHC = H // 128             # 2 h chunks (same for HSL)
NSC = S // 512             # 4 moving (token) chunks of 512
NCC = C // 128             # 8 output-row chunks
NCH = 2                    # moving chunks per expert (psum bank: 512 fp32)


def build(cap):
    nc = bacc.Bacc(
        "TRN2",
        target_bir_lowering=False,
        debug=False,
        enable_asserts=True,
        num_devices=NCORES,
    )
    # ---- DRAM I/O (per core), all bf16, pre-packed [128, ...] on host ----
    xt_d = nc.declare_dram_parameter("xt", [128, KC * S], BF, isOutput=False)
    xg_d = nc.declare_dram_parameter("xg", [128, KC * EPC * cap], BF,
                                     isOutput=False)
    wgu_d = nc.declare_dram_parameter("wgu", [128, EPC * 2 * KC * H], BF,
                                      isOutput=False)
    wdn_d = nc.declare_dram_parameter("wdn", [128, EPC * NHC * C], BF,
                                      isOutput=False)
    wsgu_d = nc.declare_dram_parameter("wsgu", [128, 2 * KC * HSL], BF,
                                       isOutput=False)
    wsdn_d = nc.declare_dram_parameter("wsdn", [128, NHC * C], BF,
                                       isOutput=False)
    outS_d = nc.declare_dram_parameter("outS", [C, S], BF, isOutput=True)
    outR_d = nc.declare_dram_parameter("outR", [C, EPC * cap], BF,
                                       isOutput=True)

    with tile.TileContext(nc) as tc:
        _emit(nc, tc, cap, xt_d, xg_d, wgu_d, wdn_d, wsgu_d, wsdn_d,
              outS_d, outR_d)
    nc.finalize()
    return nc


def _emit(nc, tc, cap, xt_d, xg_d, wgu_d, wdn_d, wsgu_d, wsdn_d,
          outS_d, outR_d):
    ch_w = cap // NCH      # tokens per moving chunk (fits a psum bank)
    res = tc.alloc_tile_pool(name="res", bufs=1)
    xg = res.tile([128, KC * EPC * cap], BF)
    wgu = res.tile([128, EPC * 2 * KC * H], BF)
    wdn = res.tile([128, EPC * NHC * C], BF)
    xt = res.tile([128, KC * S], BF)
    wsgu = res.tile([128, 2 * KC * HSL], BF)
    wsdn = res.tile([128, NHC * C], BF)
    hS = res.tile([128, NHC * S], BF)
    hR = [res.tile([128, NHC * cap], BF, name=f"hR{j}") for j in range(EPC)]

    xgv = xg.rearrange("p (k j t) -> p k j t", k=KC, j=EPC)
    wguv = wgu.rearrange("p (j r k h) -> p j r k h", j=EPC, r=2, k=KC)
    wdnv = wdn.rearrange("p (j hk c) -> p j hk c", j=EPC, hk=NHC)
    xtv = xt.rearrange("p (k s) -> p k s", k=KC)
    wsguv = wsgu.rearrange("p (r k h) -> p r k h", r=2, k=KC)
    wsdnv = wsdn.rearrange("p (hk c) -> p hk c", hk=NHC)
    hSv = hS.rearrange("p (hk s) -> p hk s", hk=NHC)
    hRv = [t.rearrange("p (hk t) -> p hk t", hk=NHC) for t in hR]
    xg_dv = xg_d.rearrange("p (k j t) -> p k j t", k=KC, j=EPC)
    wgu_dv = wgu_d.rearrange("p (j r k h) -> p j r k h", j=EPC, r=2, k=KC)
    xt_dv = xt_d.rearrange("p (k s) -> p k s", k=KC)

    # ---- DMA schedule. Transfers serialize per issuing queue (~330 GB/s
    # each in the model). Phase-1 expert-0 working set first, chunked to
    # match PE consumption order; expert-1 weights ride the scalar queue
    # (idle until the first psum results); the rest is balanced sync/pool
    # in order of first use.
    wgu_fv = wgu_d.rearrange("p (j r k h) -> p j r k h", j=EPC, r=2, k=KC)
    nc.sync.dma_start(wguv[:, 0, 0, :KC // 2], wgu_fv[:, 0, 0, :KC // 2])
    for k in range(KC):
        nc.gpsimd.dma_start(xgv[:, k, 0], xg_dv[:, k, 0])
    nc.sync.dma_start(wguv[:, 0, 0, KC // 2:], wgu_fv[:, 0, 0, KC // 2:])
    nc.sync.dma_start(wguv[:, 0, 1, :KC // 2], wgu_fv[:, 0, 1, :KC // 2])
    nc.sync.dma_start(wguv[:, 0, 1, KC // 2:], wgu_fv[:, 0, 1, KC // 2:])
    nc.scalar.dma_start(wguv[:, 1, 0], wgu_dv[:, 1, 0])
    nc.scalar.dma_start(wguv[:, 1, 1], wgu_dv[:, 1, 1])
    for k in range(KC):
        (nc.sync if k % 2 == 0 else nc.gpsimd).dma_start(
            xgv[:, k, 1], xg_dv[:, k, 1])
    nc.sync.dma_start(wsgu[:], wsgu_d[:])
    nc.gpsimd.dma_start(wdn[:], wdn_d[:])
    for k in range(KC):
        eng = nc.sync if k < KC // 2 else nc.gpsimd
        eng.dma_start(xtv[:, k], xt_dv[:, k])
    nc.gpsimd.dma_start(wsdn[:], wsdn_d[:])

    # Persistent PSUM pools spanning all phases (all 8 banks; no per-phase
    # scope-close barriers): pg/pu double-buffered for gate/up, po 4-deep
    # for the down projections.
    ppg = tc.alloc_tile_pool(name="ppg", bufs=2, space="PSUM")
    ppu = tc.alloc_tile_pool(name="ppu", bufs=2, space="PSUM")
    ppo = tc.alloc_tile_pool(name="ppo", bufs=4, space="PSUM")
    stmp = tc.alloc_tile_pool(name="stmp", bufs=3)
    sout = tc.alloc_tile_pool(name="sout", bufs=2)

    # ---------------- phase 1: routed gate/up -> hR ----------------
    for j in range(EPC):
        for hc in range(NHC):
            for ch in range(NCH):
                pg = ppg.tile([128, 512], F32, tag="pg")
                pu = ppu.tile([128, 512], F32, tag="pu")
                mv = [xgv[:, k, j, ch * ch_w:(ch + 1) * ch_w]
                      for k in range(KC)]
                hsl = slice(hc * 128, (hc + 1) * 128)
                for k in range(KC):
                    nc.tensor.matmul(pg[:, :ch_w], wguv[:, j, 0, k, hsl],
                                     mv[k],
                                     start=(k == 0), stop=(k == KC - 1))
                for k in range(KC):
                    nc.tensor.matmul(pu[:, :ch_w], wguv[:, j, 1, k, hsl],
                                     mv[k],
                                     start=(k == 0), stop=(k == KC - 1))
                tmp = stmp.tile([128, ch_w], BF, tag="t1")
                nc.scalar.activation(tmp[:], pg[:, :ch_w], AF.Silu)
                nc.vector.tensor_mul(
                    hRv[j][:, hc, ch * ch_w:(ch + 1) * ch_w],
                    tmp[:], pu[:, :ch_w])

    # ---------------- phase 2: routed down -> outR ----------------
    for cc in range(NCC):
        orr = sout.tile([128, EPC * cap], BF, tag="or")
        cs = slice(cc * 128, (cc + 1) * 128)
        for j in range(EPC):
            for ch in range(NCH):
                po = ppo.tile([128, 512], F32, tag="po")
                for hk in range(NHC):
                    nc.tensor.matmul(
                        po[:, :ch_w], wdnv[:, j, hk, cs],
                        hRv[j][:, hk, ch * ch_w:(ch + 1) * ch_w],
                        start=(hk == 0), stop=(hk == NHC - 1))
                dst = orr[:, j * cap + ch * ch_w: j * cap + (ch + 1) * ch_w]
                if (j * NCH + ch) % 2 == 0:
                    nc.scalar.copy(dst, po[:, :ch_w])
                else:
                    nc.vector.tensor_copy(dst, po[:, :ch_w])
        eng = nc.sync if cc % 2 == 0 else nc.gpsimd
        eng.dma_start(outR_d[cs, :], orr[:])

    # ---------------- phase 3: shared gate/up -> hS ----------------
    for hc in range(NHC):
        for sc in range(NSC):
            pg = ppg.tile([128, 512], F32, tag="pg")
            pu = ppu.tile([128, 512], F32, tag="pu")
            hsl = slice(hc * 128, (hc + 1) * 128)
            ss = slice(sc * 512, (sc + 1) * 512)
            for k in range(KC):
                nc.tensor.matmul(pg[:], wsguv[:, 0, k, hsl], xtv[:, k, ss],
                                 start=(k == 0), stop=(k == KC - 1))
            for k in range(KC):
                nc.tensor.matmul(pu[:], wsguv[:, 1, k, hsl], xtv[:, k, ss],
                                 start=(k == 0), stop=(k == KC - 1))
            tmp = stmp.tile([128, 512], BF, tag="t3")
            nc.scalar.activation(tmp[:], pg[:], AF.Silu)
            nc.vector.tensor_mul(hSv[:, hc, ss], tmp[:], pu[:])

    # ---------------- phase 4: shared down -> outS ----------------
    for cc in range(NCC):
        osr = sout.tile([128, S], BF, tag="os")
        cs = slice(cc * 128, (cc + 1) * 128)
        # the final row's last 512 columns are computed as two independent
        # 256-col chunks (own psum tiles / copy engines / DMA queues) so
        # the post-last-matmul tail is as short as possible
        chunks = ([(sc * 512, 512) for sc in range(NSC)] if cc < NCC - 1
                  else [(0, 512), (512, 512), (1024, 512),
                        (1536, 256), (1792, 256)])
        for i, (off, w) in enumerate(chunks):
            po = ppo.tile([128, 512], F32, tag="po")
            ss = slice(off, off + w)
            for hk in range(NHC):
                nc.tensor.matmul(po[:, :w], wsdnv[:, hk, cs], hSv[:, hk, ss],
                                 start=(hk == 0), stop=(hk == NHC - 1))
            if i % 2 == 0:
                nc.scalar.copy(osr[:, ss], po[:, :w])
            else:
                nc.vector.tensor_copy(osr[:, ss], po[:, :w])
            # stream the output row out as its chunks complete
            if cc < NCC - 1:
                if off == 512:
                    nc.sync.dma_start(outS_d[cs, :1024], osr[:, :1024])
                elif off == 1536:
                    nc.gpsimd.dma_start(outS_d[cs, 1024:], osr[:, 1024:])
            else:
                eng = nc.sync if i % 2 == 0 else nc.gpsimd
                eng.dma_start(outS_d[cs, ss], osr[:, ss])

    sout.release()
    stmp.release()
    ppo.release()
    ppu.release()
    ppg.release()
    res.release()


_NC_CACHE = {}


def _get_nc(cap):
    if cap not in _NC_CACHE:
        _NC_CACHE[cap] = build(cap)
    return _NC_CACHE[cap]


def _route_host(xf, router_w, correction_bias):
    """Replicates reference._route in float64 numpy (stable argsort matches
    jax.lax.top_k's lower-index-wins tie-breaking)."""
    x64 = xf.astype(np.float64)
    logits = x64 @ router_w.astype(np.float64).T           # [S, E]
    scores = 1.0 / (1.0 + np.exp(-logits))
    sb = scores + correction_bias.astype(np.float64)
    n = sb.shape[0]
    sbg = sb.reshape(n, G, EPG)
    grp_top = -np.sort(-sbg, axis=-1)[:, :, :PER_GROUP_K]
    group_scores = grp_top.sum(axis=-1)                    # [S, G]
    gidx = np.argsort(-group_scores, kind="stable", axis=-1)[:, :TOPK_GROUP]
    gmask = np.zeros((n, G))
    np.put_along_axis(gmask, gidx, 1.0, axis=-1)
    smask = np.repeat(gmask, EPG, axis=1)
    masked = np.where(smask > 0, sb, -np.inf)
    tk = np.argsort(-masked, kind="stable", axis=-1)[:, :TOPK]   # [S, K]
    wv = np.take_along_axis(scores, tk, axis=1)
    wv = wv / (wv.sum(axis=-1, keepdims=True) + 1e-20)
    return tk, wv


def _expert_token_lists(tk, wv):
    """Per expert: (token idx ascending, combine weights)."""
    out = []
    for e in range(E):
        tok, slot = np.nonzero(tk == e)
        out.append((tok, wv[tok, slot]))
    return out


def _pack_contract(a):
    """[C_like, F] -> [128, (kc F)] with row index c = k*128 + p."""
    ck, f = a.shape
    kc = ck // 128
    return np.ascontiguousarray(
        a.reshape(kc, 128, f).transpose(1, 0, 2).reshape(128, kc * f))


def make_in_maps(x, router_w, correction_bias, gate_w, up_w, down_w,
                 shared_gate_w, shared_up_w, shared_down_w):
    x = np.asarray(x, dtype=np.float32)
    xf = x.reshape(S, C)
    tk, wv = _route_host(xf, np.asarray(router_w, np.float32),
                         np.asarray(correction_bias, np.float32))
    experts = _expert_token_lists(tk, wv)
    # capacity = this call's max per-expert load, rounded up to even
    cap = max(2, max(len(tok) for tok, _ in experts))
    cap += cap % 2

    xT_bf = xf.T.astype(BF_NP)                              # [C, S]
    xt_pack = _pack_contract(xT_bf)                         # [128, KC*S]

    gate_w = np.asarray(gate_w, np.float32)
    up_w = np.asarray(up_w, np.float32)
    down_w = np.asarray(down_w, np.float32)
    sgT = np.asarray(shared_gate_w, np.float32).T           # [C, HS]
    suT = np.asarray(shared_up_w, np.float32).T             # [C, HS]
    sdT = np.asarray(shared_down_w, np.float32).T           # [HS, C]

    in_maps = []
    for c in range(NCORES):
        es = [c * EPC + j for j in range(EPC)]
        hs = slice(c * HSL, (c + 1) * HSL)

        # gathered tokens [128, (k j t)]
        xg = np.zeros((128, KC, EPC, cap), BF_NP)
        for j, e in enumerate(es):
            tok, _w = experts[e]
            xsel = xf[tok].T.astype(BF_NP)                  # [C, n]
            xg[:, :, j, :len(tok)] = (
                xsel.reshape(KC, 128, len(tok)).transpose(1, 0, 2))
        # routed gate/up [128, (j r k h)]
        wgu = np.stack(
            [np.stack([_pack_contract(gate_w[e].astype(BF_NP)),
                       _pack_contract(up_w[e].astype(BF_NP))], 1)
             for e in es], 1)                               # [128, j, 2, KC*H]
        # routed down [128, (j hk c)]
        wdn = np.stack([_pack_contract(down_w[e].astype(BF_NP))
                        for e in es], 1)
        wsgu = np.stack([_pack_contract(sgT[:, hs].astype(BF_NP)),
                         _pack_contract(suT[:, hs].astype(BF_NP))], 1)
        wsdn = _pack_contract(sdT[hs, :].astype(BF_NP))

        in_maps.append({
            "xt": xt_pack,
            "xg": np.ascontiguousarray(xg.reshape(128, KC * EPC * cap)),
            "wgu": np.ascontiguousarray(wgu.reshape(128, -1)),
            "wdn": np.ascontiguousarray(wdn.reshape(128, -1)),
            "wsgu": np.ascontiguousarray(wsgu.reshape(128, -1)),
            "wsdn": np.ascontiguousarray(wsdn),
        })
    return in_maps, experts, cap


def postprocess(results, experts, cap):
    accT = np.zeros((C, S), np.float64)
    for c in range(NCORES):
        accT += np.asarray(results[c]["outS"]).astype(np.float64)
        outR = np.asarray(results[c]["outR"]).astype(np.float64)
        for j in range(EPC):
            tok, w = experts[c * EPC + j]
            accT[:, tok] += outR[:, j * cap: j * cap + len(tok)] * w[None, :]
    return np.ascontiguousarray(accT.T).astype(np.float32).reshape(B, T, C)


def kernel(x, router_w, correction_bias, gate_w, up_w, down_w,
           shared_gate_w, shared_up_w, shared_down_w):
    in_maps, experts, cap = make_in_maps(
        x, router_w, correction_bias, gate_w, up_w, down_w,
        shared_gate_w, shared_up_w, shared_down_w)
    nc = _get_nc(cap)
    res = run_bass_kernel_spmd(nc, in_maps, list(range(NCORES)))
    return postprocess(res.results, experts, cap)


# revision 31
# speedup vs baseline: 2.4203x; 1.0112x over previous
"""MoE FFN (grouped sigmoid top-k routing + shared expert) on 8 TRN2 NeuronCores.

Strategy: expert-parallel with SPARSE token dispatch. Routing is computed on
the host (it determines the sharding itself — this harness's stand-in for the
"all-to-all token dispatch after routing" in the sharding hint): each core
gets 2 of 16 routed experts and receives only the tokens routed to them
(gathered + padded to a per-call capacity CAP per expert), plus 1/8 of the
shared expert (sharded along its hidden dim HS) over all tokens. All device
matmuls run in bf16 (rel-err budget 2e-2; bf16 lands ~5e-3), halving DMA and
SBUF versus fp32r at the same PE rate. Each core writes a dense shared-expert
partial [C, S] and its two experts' gathered outputs [C, CAP] (un-weighted);
the host applies the combine weights, scatter-adds, reduces over cores, and
transposes back.

CAP is sized to the call's actual max per-expert load (rounded up to even),
so no tokens are ever dropped; the compiled kernel is cached per CAP.
Device work per core: 2*CAP*3*C*H (routed) + 3*S*C*HS/8 (shared) MACs
~ 2.4 G MACs -> ~63 us PE at 2.4 GHz bf16, vs the dense-dispatch baseline's
~166 us.
"""

import numpy as np
import ml_dtypes

import concourse.bacc as bacc
import concourse.mybir as mybir
from concourse import tile
from concourse.bass_utils import run_bass_kernel_spmd

F32 = mybir.dt.float32
BF = mybir.dt.bfloat16
AF = mybir.ActivationFunctionType
BF_NP = ml_dtypes.bfloat16

# problem shapes (hardcoded; kernel.py must be self-contained)
B, T, C, H, HS = 2, 1024, 1024, 256, 2048
E, G, EPG = 16, 4, 4
TOPK, TOPK_GROUP = 4, 2
PER_GROUP_K = TOPK // TOPK_GROUP
NCORES = 8
S = B * T                  # 2048 tokens
EPC = E // NCORES          # 2 experts per core
HSL = HS // NCORES         # 256 shared-hidden rows per core
KC = C // 128              # 8 contraction chunks
NHC = H // 128             # 2 h chunks (same for HSL)
NSC = S // 512             # 4 moving (token) chunks of 512
NCC = C // 128             # 8 output-row chunks
NCH = 2                    # moving chunks per expert (psum bank: 512 fp32)


def build(caps):
    cap0, cap1 = caps
    tcap = cap0 + cap1
    nc = bacc.Bacc(
        "TRN2",
        target_bir_lowering=False,
        debug=False,
        enable_asserts=True,
        num_devices=NCORES,
    )
    # ---- DRAM I/O (per core), all bf16, pre-packed [128, ...] on host ----
    xt_d = nc.declare_dram_parameter("xt", [128, KC * S], BF, isOutput=False)
    xg_d = nc.declare_dram_parameter("xg", [128, KC * tcap], BF,
                                     isOutput=False)
    wgu_d = nc.declare_dram_parameter("wgu", [128, EPC * 2 * KC * H], BF,
                                      isOutput=False)
    wdn_d = nc.declare_dram_parameter("wdn", [128, EPC * NHC * C], BF,
                                      isOutput=False)
    wsgu_d = nc.declare_dram_parameter("wsgu", [128, 2 * KC * HSL], BF,
                                       isOutput=False)
    wsdn_d = nc.declare_dram_parameter("wsdn", [128, NHC * C], BF,
                                       isOutput=False)
    outS_d = nc.declare_dram_parameter("outS", [C, S], BF, isOutput=True)
    outR_d = nc.declare_dram_parameter("outR", [C, tcap], BF,
                                       isOutput=True)

    with tile.TileContext(nc) as tc:
        _emit(nc, tc, caps, xt_d, xg_d, wgu_d, wdn_d, wsgu_d, wsdn_d,
              outS_d, outR_d)
    nc.finalize()
    return nc


def _emit(nc, tc, caps, xt_d, xg_d, wgu_d, wdn_d, wsgu_d, wsdn_d,
          outS_d, outR_d):
    # per-slot moving-chunk lists (each chunk must fit a 512-fp32 psum bank)
    xoff = [0, caps[0]]
    tcap = caps[0] + caps[1]
    chunks = []
    for cp in caps:
        if cp <= 512:
            chunks.append([(0, cp)])
        else:
            h = (cp // 2 + 1) // 2 * 2
            chunks.append([(0, h), (h, cp - h)])
    res = tc.alloc_tile_pool(name="res", bufs=1)
    xg = res.tile([128, KC * tcap], BF)
    wgu = res.tile([128, EPC * 2 * KC * H], BF)
    wdn = res.tile([128, EPC * NHC * C], BF)
    xt = res.tile([128, KC * S], BF)
    wsgu = res.tile([128, 2 * KC * HSL], BF)
    wsdn = res.tile([128, NHC * C], BF)
    hS = res.tile([128, NHC * S], BF)
    hR = [res.tile([128, NHC * caps[j]], BF, name=f"hR{j}")
          for j in range(EPC)]

    xgv = xg.rearrange("p (k t) -> p k t", k=KC)
    wguv = wgu.rearrange("p (j r k h) -> p j r k h", j=EPC, r=2, k=KC)
    wdnv = wdn.rearrange("p (j hk c) -> p j hk c", j=EPC, hk=NHC)
    xtv = xt.rearrange("p (k s) -> p k s", k=KC)
    wsguv = wsgu.rearrange("p (r k h) -> p r k h", r=2, k=KC)
    wsdnv = wsdn.rearrange("p (hk c) -> p hk c", hk=NHC)
    hSv = hS.rearrange("p (hk s) -> p hk s", hk=NHC)
    hRv = [t.rearrange("p (hk t) -> p hk t", hk=NHC) for t in hR]
    xg_dv = xg_d.rearrange("p (k t) -> p k t", k=KC)
    wgu_dv = wgu_d.rearrange("p (j r k h) -> p j r k h", j=EPC, r=2, k=KC)
    xt_dv = xt_d.rearrange("p (k s) -> p k s", k=KC)

    # ---- DMA schedule. Transfers serialize per issuing queue (~330 GB/s
    # each in the model). Phase-1 expert-0 working set first, chunked to
    # match PE consumption order; expert-1 weights ride the scalar queue
    # (idle until the first psum results); the rest is balanced sync/pool
    # in order of first use.
    wgu_fv = wgu_d.rearrange("p (j r k h) -> p j r k h", j=EPC, r=2, k=KC)
    nc.sync.dma_start(wguv[:, 0, 0, :KC // 2], wgu_fv[:, 0, 0, :KC // 2])
    for k in range(KC):
        nc.gpsimd.dma_start(xgv[:, k, :caps[0]], xg_dv[:, k, :caps[0]])
    nc.sync.dma_start(wguv[:, 0, 0, KC // 2:], wgu_fv[:, 0, 0, KC // 2:])
    nc.sync.dma_start(wguv[:, 0, 1, :KC // 2], wgu_fv[:, 0, 1, :KC // 2])
    nc.sync.dma_start(wguv[:, 0, 1, KC // 2:], wgu_fv[:, 0, 1, KC // 2:])
    nc.scalar.dma_start(wguv[:, 1, 0], wgu_dv[:, 1, 0])
    nc.scalar.dma_start(wguv[:, 1, 1], wgu_dv[:, 1, 1])
    for k in range(KC):
        (nc.sync if k % 2 == 0 else nc.gpsimd).dma_start(
            xgv[:, k, caps[0]:], xg_dv[:, k, caps[0]:])
    nc.sync.dma_start(wsgu[:], wsgu_d[:])
    nc.gpsimd.dma_start(wdn[:], wdn_d[:])
    for k in range(KC):
        eng = nc.sync if k < KC // 2 else nc.gpsimd
        eng.dma_start(xtv[:, k], xt_dv[:, k])
    nc.gpsimd.dma_start(wsdn[:], wsdn_d[:])

    # Persistent PSUM pools spanning all phases (all 8 banks; no per-phase
    # scope-close barriers): pg/pu double-buffered for gate/up, po 4-deep
    # for the down projections.
    ppg = tc.alloc_tile_pool(name="ppg", bufs=2, space="PSUM")
    ppu = tc.alloc_tile_pool(name="ppu", bufs=2, space="PSUM")
    ppo = tc.alloc_tile_pool(name="ppo", bufs=4, space="PSUM")
    stmp = tc.alloc_tile_pool(name="stmp", bufs=3)
    sout = tc.alloc_tile_pool(name="sout", bufs=4)

    # ---------------- phase 1: routed gate/up -> hR ----------------
    for j in range(EPC):
        for hc in range(NHC):
            for (off, w) in chunks[j]:
                pg = ppg.tile([128, 512], F32, tag="pg")
                pu = ppu.tile([128, 512], F32, tag="pu")
                mv = [xgv[:, k, xoff[j] + off: xoff[j] + off + w]
                      for k in range(KC)]
                hsl = slice(hc * 128, (hc + 1) * 128)
                for k in range(KC):
                    nc.tensor.matmul(pg[:, :w], wguv[:, j, 0, k, hsl],
                                     mv[k],
                                     start=(k == 0), stop=(k == KC - 1))
                for k in range(KC):
                    nc.tensor.matmul(pu[:, :w], wguv[:, j, 1, k, hsl],
                                     mv[k],
                                     start=(k == 0), stop=(k == KC - 1))
                tmp = stmp.tile([128, 512], BF, tag="t1")
                nc.scalar.activation(tmp[:, :w], pg[:, :w], AF.Silu)
                nc.vector.tensor_mul(
                    hRv[j][:, hc, off:off + w],
                    tmp[:, :w], pu[:, :w])

    # ---------------- phase 2: routed down -> outR ----------------
    for cc in range(NCC):
        orr = sout.tile([128, tcap], BF, tag="or")
        cs = slice(cc * 128, (cc + 1) * 128)
        ci = 0
        for j in range(EPC):
            for (off, w) in chunks[j]:
                po = ppo.tile([128, 512], F32, tag="po")
                for hk in range(NHC):
                    nc.tensor.matmul(
                        po[:, :w], wdnv[:, j, hk, cs],
                        hRv[j][:, hk, off:off + w],
                        start=(hk == 0), stop=(hk == NHC - 1))
                dst = orr[:, xoff[j] + off: xoff[j] + off + w]
                if ci % 2 == 0:
                    nc.scalar.copy(dst, po[:, :w])
                else:
                    nc.vector.tensor_copy(dst, po[:, :w])
                ci += 1
        eng = nc.sync if cc % 2 == 0 else nc.gpsimd
        eng.dma_start(outR_d[cs, :], orr[:])

    # ---------------- phase 3: shared gate/up -> hS ----------------
    for hc in range(NHC):
        for sc in range(NSC):
            pg = ppg.tile([128, 512], F32, tag="pg")
            pu = ppu.tile([128, 512], F32, tag="pu")
            hsl = slice(hc * 128, (hc + 1) * 128)
            ss = slice(sc * 512, (sc + 1) * 512)
            for k in range(KC):
                nc.tensor.matmul(pg[:], wsguv[:, 0, k, hsl], xtv[:, k, ss],
                                 start=(k == 0), stop=(k == KC - 1))
            for k in range(KC):
                nc.tensor.matmul(pu[:], wsguv[:, 1, k, hsl], xtv[:, k, ss],
                                 start=(k == 0), stop=(k == KC - 1))
            tmp = stmp.tile([128, 512], BF, tag="t3")
            nc.scalar.activation(tmp[:], pg[:], AF.Silu)
            nc.vector.tensor_mul(hSv[:, hc, ss], tmp[:], pu[:])

    # ---------------- phase 4: shared down -> outS ----------------
    for cc in range(NCC):
        osr = sout.tile([128, S], BF, tag="os")
        cs = slice(cc * 128, (cc + 1) * 128)
        # the final row's last 512 columns are computed as two independent
        # 256-col chunks (own psum tiles / copy engines / DMA queues) so
        # the post-last-matmul tail is as short as possible
        chunks = ([(sc * 512, 512) for sc in range(NSC)] if cc < NCC - 1
                  else [(0, 512), (512, 512), (1024, 512),
                        (1536, 256), (1792, 256)])
        for i, (off, w) in enumerate(chunks):
            po = ppo.tile([128, 512], F32, tag="po")
            ss = slice(off, off + w)
            for hk in range(NHC):
                nc.tensor.matmul(po[:, :w], wsdnv[:, hk, cs], hSv[:, hk, ss],
                                 start=(hk == 0), stop=(hk == NHC - 1))
            if i % 2 == 0:
                nc.scalar.copy(osr[:, ss], po[:, :w])
            else:
                nc.vector.tensor_copy(osr[:, ss], po[:, :w])
            # stream the output row out as its chunks complete
            if cc < NCC - 1:
                if off == 512:
                    nc.sync.dma_start(outS_d[cs, :1024], osr[:, :1024])
                elif off == 1536:
                    nc.gpsimd.dma_start(outS_d[cs, 1024:], osr[:, 1024:])
            else:
                eng = nc.sync if i % 2 == 0 else nc.gpsimd
                eng.dma_start(outS_d[cs, ss], osr[:, ss])

    sout.release()
    stmp.release()
    ppo.release()
    ppu.release()
    ppg.release()
    res.release()


_NC_CACHE = {}


def _get_nc(caps):
    if caps not in _NC_CACHE:
        _NC_CACHE[caps] = build(caps)
    return _NC_CACHE[caps]


def _route_host(xf, router_w, correction_bias):
    """Replicates reference._route in float64 numpy (stable argsort matches
    jax.lax.top_k's lower-index-wins tie-breaking)."""
    x64 = xf.astype(np.float64)
    logits = x64 @ router_w.astype(np.float64).T           # [S, E]
    scores = 1.0 / (1.0 + np.exp(-logits))
    sb = scores + correction_bias.astype(np.float64)
    n = sb.shape[0]
    sbg = sb.reshape(n, G, EPG)
    grp_top = -np.sort(-sbg, axis=-1)[:, :, :PER_GROUP_K]
    group_scores = grp_top.sum(axis=-1)                    # [S, G]
    gidx = np.argsort(-group_scores, kind="stable", axis=-1)[:, :TOPK_GROUP]
    gmask = np.zeros((n, G))
    np.put_along_axis(gmask, gidx, 1.0, axis=-1)
    smask = np.repeat(gmask, EPG, axis=1)
    masked = np.where(smask > 0, sb, -np.inf)
    tk = np.argsort(-masked, kind="stable", axis=-1)[:, :TOPK]   # [S, K]
    wv = np.take_along_axis(scores, tk, axis=1)
    wv = wv / (wv.sum(axis=-1, keepdims=True) + 1e-20)
    return tk, wv


def _expert_token_lists(tk, wv):
    """Per expert: (token idx ascending, combine weights)."""
    out = []
    for e in range(E):
        tok, slot = np.nonzero(tk == e)
        out.append((tok, wv[tok, slot]))
    return out


def _assign_experts(experts):
    """(core, slot) -> expert id. Slot 0 takes the 8 most-loaded experts,
    slot 1 the rest, so slot 1's capacity (and its padding) is smaller."""
    order = np.argsort([-len(tok) for tok, _ in experts], kind="stable")
    assign = np.zeros((NCORES, EPC), np.int64)
    caps = []
    for j in range(EPC):
        grp = order[j * NCORES:(j + 1) * NCORES]
        assign[:, j] = grp
        cp = max(2, max(len(experts[e][0]) for e in grp))
        caps.append(cp + cp % 2)
    return assign, tuple(caps)


def _pack_contract(a):
    """[C_like, F] -> [128, (kc F)] with row index c = k*128 + p."""
    ck, f = a.shape
    kc = ck // 128
    return np.ascontiguousarray(
        a.reshape(kc, 128, f).transpose(1, 0, 2).reshape(128, kc * f))


def make_in_maps(x, router_w, correction_bias, gate_w, up_w, down_w,
                 shared_gate_w, shared_up_w, shared_down_w):
    x = np.asarray(x, dtype=np.float32)
    xf = x.reshape(S, C)
    tk, wv = _route_host(xf, np.asarray(router_w, np.float32),
                         np.asarray(correction_bias, np.float32))
    experts = _expert_token_lists(tk, wv)
    assign, caps = _assign_experts(experts)
    xoff = [0, caps[0]]
    tcap = caps[0] + caps[1]

    xT_bf = xf.T.astype(BF_NP)                              # [C, S]
    xt_pack = _pack_contract(xT_bf)                         # [128, KC*S]

    gate_w = np.asarray(gate_w, np.float32)
    up_w = np.asarray(up_w, np.float32)
    down_w = np.asarray(down_w, np.float32)
    sgT = np.asarray(shared_gate_w, np.float32).T           # [C, HS]
    suT = np.asarray(shared_up_w, np.float32).T             # [C, HS]
    sdT = np.asarray(shared_down_w, np.float32).T           # [HS, C]

    in_maps = []
    for c in range(NCORES):
        es = [int(assign[c, j]) for j in range(EPC)]
        hs = slice(c * HSL, (c + 1) * HSL)

        # gathered tokens [128, (k t)], slot layout [slot0 | slot1]
        xg = np.zeros((128, KC, tcap), BF_NP)
        for j, e in enumerate(es):
            tok, _w = experts[e]
            xsel = xf[tok].T.astype(BF_NP)                  # [C, n]
            xg[:, :, xoff[j]:xoff[j] + len(tok)] = (
                xsel.reshape(KC, 128, len(tok)).transpose(1, 0, 2))
        # routed gate/up [128, (j r k h)]
        wgu = np.stack(
            [np.stack([_pack_contract(gate_w[e].astype(BF_NP)),
                       _pack_contract(up_w[e].astype(BF_NP))], 1)
             for e in es], 1)                               # [128, j, 2, KC*H]
        # routed down [128, (j hk c)]
        wdn = np.stack([_pack_contract(down_w[e].astype(BF_NP))
                        for e in es], 1)
        wsgu = np.stack([_pack_contract(sgT[:, hs].astype(BF_NP)),
                         _pack_contract(suT[:, hs].astype(BF_NP))], 1)
        wsdn = _pack_contract(sdT[hs, :].astype(BF_NP))

        in_maps.append({
            "xt": xt_pack,
            "xg": np.ascontiguousarray(xg.reshape(128, KC * tcap)),
            "wgu": np.ascontiguousarray(wgu.reshape(128, -1)),
            "wdn": np.ascontiguousarray(wdn.reshape(128, -1)),
            "wsgu": np.ascontiguousarray(wsgu.reshape(128, -1)),
            "wsdn": np.ascontiguousarray(wsdn),
        })
    return in_maps, (experts, assign), caps


def postprocess(results, routing, caps):
    experts, assign = routing
    xoff = [0, caps[0]]
    accT = np.zeros((C, S), np.float64)
    for c in range(NCORES):
        accT += np.asarray(results[c]["outS"]).astype(np.float64)
        outR = np.asarray(results[c]["outR"]).astype(np.float64)
        for j in range(EPC):
            tok, w = experts[int(assign[c, j])]
            accT[:, tok] += (outR[:, xoff[j]: xoff[j] + len(tok)]
                             * w[None, :])
    return np.ascontiguousarray(accT.T).astype(np.float32).reshape(B, T, C)


def kernel(x, router_w, correction_bias, gate_w, up_w, down_w,
           shared_gate_w, shared_up_w, shared_down_w):
    in_maps, routing, caps = make_in_maps(
        x, router_w, correction_bias, gate_w, up_w, down_w,
        shared_gate_w, shared_up_w, shared_down_w)
    nc = _get_nc(caps)
    res = run_bass_kernel_spmd(nc, in_maps, list(range(NCORES)))
    return postprocess(res.results, routing, caps)
